# revision 35
# baseline (speedup 1.0000x reference)
"""Distributed Trainium2 Bass kernel for causal multi-head attention w/ RoPE.

Problem shapes (hardcoded): B=2, S=2048, D=1024, H=16, HD=64.
Sharding: tensor-parallel over heads — each of 8 cores owns 2 heads
(column slice of wq/wk/wv, row slice of wo). Each core emits its partial
x @ woT contribution; the host sums the 8 partials (the "all-reduce").

Schedule (v3): token-block-outer pipeline. For each 512-token block:
q/k/v projections (bf16 PE, fp32 PSUM), RoPE (PE block-swap matmul + DVE
mul/mul/add), v-transpose into v' = [v | 1] tiles. As soon as block g of
batch b is done, attention group (b, g) runs: per sk-tile, BOTH heads'
scores land in one [128,1024] PSUM tile (h0 cols 0:512, h1 512:1024),
the -1e9 causal mask is accumulated on the PE (tri matmul) for diagonal
tiles, ONE wide exp on ScalarE covers both heads, then two PV matmuls
accumulate [out | denom] per head.

Softmax normalization (hardware-profiled choices): 1/d comes from the
fp32 bit-trick seed (magic - bits(d), as an int32 tensor_scalar) plus a
Newton step shaped as (d*r0 - 2)*r0 = -1/d; woT is host-negated to absorb
the sign. nc.vector.reciprocal costs ~4us per row on HW and
reciprocal_approx_fast returns garbage via this runtime, and Ln/Exp sit
in different activation tables (1.3us reload each way), so DVE int ops
are the only cheap correct path. gpsimd executes ONLY
partition_broadcast: every distinct gpsimd op type swaps in its own Q7
library at ~7.5us per reload, which serialized the whole kernel when
broadcasts, tensor ops and DMA triggers shared the engine. head-1's
normalized tile loopback-DMAs into outT partitions 64:128 (engines
cannot write across partition offsets; DMA can). wo output-chunk matmuls
follow each group; fp32 PSUM is cast to bf16 by DVE/ScalarE copies
(alternating so two casts drain concurrently) and DMA'd out. All bulk
DMA rides sync+scalar queues: every dma_start costs ~0.6-1.0us of issue
time on the issuing engine's sequencer, so gpsimd/vector stay clean for
compute and first-needed tiles alternate between the two queues.
"""

import sys

sys.path.insert(0, "/opt/trn_rl_repo")

import numpy as np
import ml_dtypes

B, S, D, H = 2, 2048, 1024, 16
HD = D // H  # 64
NC = 8
HPC = H // NC  # heads per core = 2
HDC = HPC * HD  # head dims per core = 128
TOK = B * S  # 4096
BF16 = ml_dtypes.bfloat16

_COMPILED = {}


def _build_program():
    import concourse.bass as bass
    import concourse.mybir as mybir
    import concourse.bacc as bacc
    from concourse import tile

    f32 = mybir.dt.float32
    bf16 = mybir.dt.bfloat16
    MULT = mybir.AluOpType.mult
    ADD = mybir.AluOpType.add
    DIV = mybir.AluOpType.divide
    EXP = mybir.ActivationFunctionType.Exp
    LN = mybir.ActivationFunctionType.Ln

    nc = bacc.Bacc("TRN2", target_bir_lowering=False, debug=False, num_devices=NC)

    KT = D // 128  # 8 contraction tiles for projections
    NTB = TOK // 512  # 8 tok blocks of 512
    NG = S // 512  # 4 groups per batch

    xT_d = nc.dram_tensor("xT", [NTB * KT * 128, 512], bf16,
                          kind="ExternalInput").ap()
    wqT_d = nc.dram_tensor("wqT", [D, HDC], bf16, kind="ExternalInput").ap()
    wkT_d = nc.dram_tensor("wkT", [D, HDC], bf16, kind="ExternalInput").ap()
    wvT_d = nc.dram_tensor("wvT", [D, HDC], bf16, kind="ExternalInput").ap()
    woT_d = nc.dram_tensor("woT", [HDC, D], bf16, kind="ExternalInput").ap()
    PT_d = nc.dram_tensor("PT", [HDC, HDC], bf16, kind="ExternalInput").ap()
    cos_d = nc.dram_tensor("cosx", [HDC, S], bf16, kind="ExternalInput").ap()
    sin_d = nc.dram_tensor("sinx", [HDC, S], bf16, kind="ExternalInput").ap()
    tri_d = nc.dram_tensor("tri", [128, 128], bf16, kind="ExternalInput").ap()
    id_d = nc.dram_tensor("ident", [128, 128], bf16, kind="ExternalInput").ap()
    out_d = nc.dram_tensor("out", [D, TOK], bf16, kind="ExternalOutput").ap()

    with tile.TileContext(nc) as tc:
        with (
            tc.tile_pool(name="big", bufs=1) as big,
            tc.tile_pool(name="work", bufs=3) as work,
            tc.tile_pool(name="etp", bufs=10) as etp,
            tc.tile_pool(name="nrm", bufs=3) as nrm,
            tc.tile_pool(name="wop", bufs=6) as wop,
            tc.tile_pool(name="mmp", bufs=2, space="PSUM") as mmp,
            tc.tile_pool(name="scp", bufs=2, space="PSUM") as scp,
            tc.tile_pool(name="pop", bufs=2, space="PSUM") as pop,
        ):
            # ---- input DMAs: first-needed first -------------------------
            # DMA issue cost is ~0.7-1us per dma_start on every engine, so
            # first-needed transfers alternate between the sync and scalar
            # queues and small/late tensors ride scalar (idle early).
            wq = big.tile([128, KT * HDC], bf16, tag="wq")
            wk = big.tile([128, KT * HDC], bf16, tag="wk")
            wv = big.tile([128, KT * HDC], bf16, tag="wv")
            xT = big.tile([128, KT * TOK], bf16, tag="xT")

            def xt_dma(tb, k, eng):
                r0 = (tb * KT + k) * 128
                eng.dma_start(
                    xT[:, k * TOK + tb * 512 : k * TOK + (tb + 1) * 512],
                    xT_d[r0 : r0 + 128, :])

            for k in range(KT):  # first projection block's needs
                (nc.sync if k % 2 else nc.scalar).dma_start(
                    wq[:, k * HDC : (k + 1) * HDC],
                    wqT_d[k * 128 : (k + 1) * 128, :])
            for k in range(KT):
                xt_dma(0, k, nc.scalar if k % 2 else nc.sync)
            for w_sb, w_d in ((wk, wkT_d), (wv, wvT_d)):
                for k in range(KT):
                    nc.sync.dma_start(w_sb[:, k * HDC : (k + 1) * HDC],
                                      w_d[k * 128 : (k + 1) * 128, :])
            for tb in range(1, NTB):
                for k in range(KT):
                    xt_dma(tb, k, nc.sync)

            # scalar queue: PT/ident early (rope + transpose of block 0),
            # then cos/sin, tri, wo
            PT = big.tile([128, 128], bf16, tag="PT")
            nc.scalar.dma_start(PT[:], PT_d[:, :])
            ident = big.tile([128, 128], bf16, tag="ident")
            nc.scalar.dma_start(ident[:], id_d[:, :])
            cosx = big.tile([128, S], bf16, tag="cosx")
            nc.scalar.dma_start(cosx[:], cos_d[:, :])
            sinx = big.tile([128, S], bf16, tag="sinx")
            nc.scalar.dma_start(sinx[:], sin_d[:, :])
            tri = big.tile([128, 128], bf16, tag="tri")
            nc.scalar.dma_start(tri[:], tri_d[:, :])
            wo = big.tile([128, D], bf16, tag="wo")
            nc.scalar.dma_start(wo[:], woT_d[:, :])

            # ---- persistent SBUF state ----------------------------------
            rotq = big.tile([128, TOK], bf16, tag="rotq")
            rotk = big.tile([128, TOK], bf16, tag="rotk")
            # v' tiles: [part, kt, head, 65] with ones in col 64 (set once)
            vp = big.tile([128, TOK // 128, HPC, HD + 1], bf16, tag="vp")
            nc.gpsimd.memset(vp[:, :, :, HD : HD + 1], 1.0)
            outT = [big.tile([128, S], bf16, tag=f"outT{b}", name=f"outT{b}")
                    for b in range(B)]

            def proj_block(tb):
                """projections + RoPE + v' for token block tb (512 toks)"""
                blk = slice(tb * 512, (tb + 1) * 512)
                sblk = slice((tb % NG) * 512, (tb % NG + 1) * 512)
                sbs = []
                for w_sb, nm in ((wq, "q"), (wk, "k"), (wv, "v")):
                    ps = mmp.tile([128, 512], f32, tag="mm", name=f"ps{nm}{tb}")
                    for k in range(KT):
                        nc.tensor.matmul(
                            ps[:],
                            w_sb[:, k * HDC : (k + 1) * HDC],
                            xT[:, k * TOK + tb * 512 : k * TOK + (tb + 1) * 512],
                            start=(k == 0), stop=(k == KT - 1),
                        )
                    sb = work.tile([128, 512], bf16, tag=f"{nm}sb")
                    nc.vector.tensor_copy(sb[:], ps[:])
                    sbs.append(sb)
                qsb, ksb, vsb = sbs
                for src, rotdst in ((qsb, rotq), (ksb, rotk)):
                    pss = mmp.tile([128, 512], f32, tag="mm", name=f"pr{tb}")
                    nc.tensor.matmul(pss[:], PT[:], src[:], start=True, stop=True)
                    t1 = work.tile([128, 512], bf16, tag="t1")
                    nc.vector.tensor_tensor(t1[:], src[:], cosx[:, sblk], MULT)
                    t2 = work.tile([128, 512], bf16, tag="t2")
                    nc.vector.tensor_tensor(t2[:], pss[:], sinx[:, sblk], MULT)
                    nc.vector.tensor_tensor(rotdst[:, blk], t1[:], t2[:], ADD)
                for j in range(4):
                    gkt = tb * 4 + j
                    pst = mmp.tile([128, 128], bf16, tag="mm", name=f"pt{gkt}")
                    nc.tensor.transpose(pst[:], vsb[:, j * 128 : (j + 1) * 128],
                                        ident[:])
                    nc.vector.tensor_copy(
                        vp[:, gkt, :, 0:HD],
                        pst[:].rearrange("p (a i) -> p a i", a=HPC))

            def attention_group(b, g):
                """scores+softmax+PV for sq cols [512g, 512g+512) of batch b"""
                g0 = g * 512
                nkt = 4 * g + 4
                po = [pop.tile([HD + 1, 512], f32, tag="po",
                               name=f"po{b}{g}{h}") for h in range(HPC)]
                for kt in range(nkt):
                    w0 = kt * 128
                    lo = max(0, w0 - g0)
                    sc = scp.tile([128, 1024], f32, tag="sc",
                                  name=f"sc{b}{g}{kt}")
                    # h0: only causal cols; h1: full width (junk below diag
                    # is written, exp'd, and never read by PV)
                    diag = w0 >= g0  # diagonal tile: -1e9 tri mask gets added
                    # h1 covers full width: junk below the diagonal is
                    # written (never read by PV) so the wide exp reads no
                    # stale psum
                    nc.tensor.matmul(
                        sc[:, lo:512],
                        rotk[0:HD, b * S + w0 : b * S + w0 + 128],
                        rotq[0:HD, b * S + g0 + lo : b * S + g0 + 512],
                        start=True, stop=not diag)
                    nc.tensor.matmul(
                        sc[:, 512:1024],
                        rotk[HD : 2 * HD, b * S + w0 : b * S + w0 + 128],
                        rotq[HD : 2 * HD, b * S + g0 : b * S + g0 + 512],
                        start=True, stop=not diag)
                    if diag:
                        nc.tensor.matmul(sc[:, lo : lo + 128], ident[:],
                                         tri[:], start=False, stop=True)
                        nc.tensor.matmul(sc[:, 512 + lo : 512 + lo + 128],
                                         ident[:], tri[:],
                                         start=False, stop=True)
                    et = etp.tile([128, 1024], bf16, tag="et")
                    nc.scalar.activation(et[:, lo:1024], sc[:, lo:1024],
                                         EXP, scale=0.125)
                    nc.tensor.matmul(
                        po[0][:, lo:512], vp[:, b * (S // 128) + kt, 0, :],
                        et[:, lo:512],
                        start=(kt == 0), stop=(kt == nkt - 1))
                    nc.tensor.matmul(
                        po[1][:, lo:512], vp[:, b * (S // 128) + kt, 1, :],
                        et[:, 512 + lo : 1024],
                        start=(kt == 0), stop=(kt == nkt - 1))
                # normalization: out = po[0:64] * (1/po[64]) per column.
                # 1/d via the fp32 bit-trick seed (magic - bits(d), done as
                # NOT(bits(d)) + magic+1 in one int tensor_scalar) plus one
                # Newton step emitted as (d*r0 - 2)*r0 = -r1, so the chain
                # yields -1/d; woT is negated on the host to compensate.
                # gpsimd runs ONLY partition_broadcast (one Q7 library).
                i32 = mybir.dt.int32
                rl = [nrm.tile([1, 512], f32, tag=f"rl{h}", name=f"rl{b}{g}{h}")
                      for h in range(HPC)]
                rt = [nrm.tile([1, 512], f32, tag=f"rt{h}", name=f"rt{b}{g}{h}")
                      for h in range(HPC)]
                r = [nrm.tile([1, 512], f32, tag=f"r{h}", name=f"r{b}{g}{h}")
                     for h in range(HPC)]
                SUB = mybir.AluOpType.subtract
                for h in range(HPC):
                    nc.vector.tensor_scalar(
                        rl[h][0:1, :].bitcast(i32),
                        po[h][HD : HD + 1, :].bitcast(i32),
                        0x7EF311C3, -1, SUB, MULT)
                    nc.vector.tensor_tensor(rt[h][0:1, :],
                                            po[h][HD : HD + 1, :],
                                            rl[h][0:1, :], MULT)
                    nc.vector.scalar_tensor_tensor(
                        r[h][0:1, :], rt[h][0:1, :], 2.0, rl[h][0:1, :],
                        SUB, MULT)
                rb = [nrm.tile([HD, 512], f32, tag="rb", name=f"rb{b}{g}{h}")
                      for h in range(HPC)]
                for h in range(HPC):
                    nc.gpsimd.partition_broadcast(rb[h][:, :], r[h][0:1, :])
                nc.vector.tensor_tensor(outT[b][0:HD, g0 : g0 + 512],
                                        po[0][0:HD, :], rb[0][:, :], MULT)
                oh = nrm.tile([HD, 512], bf16, tag="oh")
                nc.vector.tensor_tensor(oh[:], po[1][0:HD, :], rb[1][:, :],
                                        MULT)
                nc.sync.dma_start(outT[b][HD : 2 * HD, g0 : g0 + 512], oh[:])

            def wo_group(b, g):
                """wo partial for out cols [512g, 512g+512) of batch b"""
                g0 = g * 512
                for o in range(D // 128):
                    psw = mmp.tile([128, 512], f32, tag="mm",
                                   name=f"pw{b}{g}{o}")
                    nc.tensor.matmul(
                        psw[:], wo[:, o * 128 : (o + 1) * 128],
                        outT[b][:, g0 : g0 + 512], start=True, stop=True)
                    wout = wop.tile([128, 512], bf16, tag="wout")
                    # batch 1's wo overlaps the last attention groups where
                    # ScalarE paces the kernel -- keep its casts on DVE
                    if o % 2 == 0 or b == 1:
                        nc.vector.tensor_copy(wout[:], psw[:])
                    else:
                        nc.scalar.copy(wout[:], psw[:])
                    nc.sync.dma_start(
                        out_d[o * 128 : (o + 1) * 128,
                              b * S + g0 : b * S + g0 + 512],
                        wout[:])

            # ---- emission order: software-pipelined schedule ------------
            proj_block(0)
            proj_block(1)
            attention_group(0, 0)
            proj_block(2)
            attention_group(0, 1)
            proj_block(3)
            attention_group(0, 2)
            wo_group(0, 0)
            proj_block(4)
            attention_group(0, 3)
            wo_group(0, 1)
            proj_block(5)
            attention_group(1, 0)
            wo_group(0, 2)
            proj_block(6)
            attention_group(1, 1)
            wo_group(0, 3)
            proj_block(7)
            attention_group(1, 2)
            wo_group(1, 0)
            attention_group(1, 3)
            wo_group(1, 1)
            wo_group(1, 2)
            wo_group(1, 3)

    nc.compile()
    return nc


def _host_inputs(x, wq, wk, wv, wo, freqs_cos, freqs_sin):
    """Build the per-core input maps (all host-side transforms are free)."""
    perm = np.concatenate([np.arange(0, HD, 2), np.arange(1, HD, 2)])  # rot-half
    xTf = x.reshape(TOK, D).T.astype(BF16)  # [D, TOK]
    # chunk-contiguous tiling: row block (tb*KT+k) holds xT[k*128:+128, tb*512:+512]
    xT = np.zeros(((TOK // 512) * (D // 128) * 128, 512), BF16)
    for tb in range(TOK // 512):
        for k in range(D // 128):
            r0 = (tb * (D // 128) + k) * 128
            xT[r0 : r0 + 128, :] = xTf[k * 128 : (k + 1) * 128,
                                       tb * 512 : (tb + 1) * 512]

    # signed block-swap P (per 64-dim head): qs_lo = -q_hi, qs_hi = q_lo
    P = np.zeros((HDC, HDC), np.float32)
    for h in range(HPC):
        base = h * HD
        half = HD // 2
        for i in range(half):
            P[base + i, base + half + i] = -1.0
            P[base + half + i, base + i] = 1.0
    PT = np.ascontiguousarray(P.T).astype(BF16)

    # cos/sin expanded to [HDC, S]; row j within a head uses freq j%32
    half = HD // 2
    idx = np.concatenate([np.arange(half), np.arange(half)])  # [64]
    cos1 = freqs_cos[:, :].T[idx]  # [64, S]
    sin1 = freqs_sin[:, :].T[idx]
    cosx = np.tile(cos1, (HPC, 1)).astype(BF16)  # [128, S]
    sinx = np.tile(sin1, (HPC, 1)).astype(BF16)

    # additive causal mask for the diagonal tile: 0 where sk<=sq, -1e9 else
    tri = np.where(np.triu(np.ones((128, 128), dtype=bool)), 0.0,
                   -1e9).astype(BF16)
    ident = np.eye(128, dtype=np.float32).astype(BF16)

    in_maps = []
    for c in range(NC):
        rows = []
        for h in range(HPC):
            hg = c * HPC + h
            rows.append(hg * HD + perm)
        rows = np.concatenate(rows)
        wq_c = np.ascontiguousarray(wq[rows, :].T).astype(BF16)  # [D, 128]
        wk_c = np.ascontiguousarray(wk[rows, :].T).astype(BF16)
        vrows = np.arange(c * HDC, (c + 1) * HDC)
        wv_c = np.ascontiguousarray(wv[vrows, :].T).astype(BF16)
        # negated: the on-device softmax scale is -1/d (sign from the
        # Newton-step formulation); two sign flips cancel in x @ woT
        wo_c = np.ascontiguousarray(-wo[:, vrows].T).astype(BF16)  # [128, D]
        in_maps.append({
            "xT": xT, "wqT": wq_c, "wkT": wk_c, "wvT": wv_c, "woT": wo_c,
            "PT": PT, "cosx": cosx, "sinx": sinx, "tri": tri,
            "ident": ident,
        })
    return in_maps


def _install_ntff_hook():
    """Provide antenv.axon_hooks (missing in this image) so that
    run_bass_kernel_spmd(trace=True) can capture an NTFF profile via the
    axon PJRT .so — replicates trn_boot._ntff_profile_via_ctypes."""
    import types, ctypes, contextlib, sys as _sys

    if "antenv.axon_hooks" in _sys.modules:
        return
    so_path = "/opt/axon/libaxon_pjrt.so"
    try:
        lib = ctypes.CDLL(so_path)
    except OSError:
        return
    if not hasattr(lib, "axon_start_nrt_profile"):
        return
    lib.axon_start_nrt_profile.argtypes = [ctypes.POINTER(ctypes.c_int64),
                                           ctypes.c_size_t]
    lib.axon_start_nrt_profile.restype = ctypes.c_int64
    lib.axon_stop_nrt_profile.argtypes = [ctypes.c_char_p]
    lib.axon_stop_nrt_profile.restype = ctypes.c_int64

    @contextlib.contextmanager
    def _hook(output_dir, device_ids):
        import jax
        jax.devices()
        if device_ids:
            ids = (ctypes.c_int64 * len(device_ids))(*device_ids)
            rc = lib.axon_start_nrt_profile(ids, len(device_ids))
        else:
            rc = lib.axon_start_nrt_profile(None, 0)
        if rc != 0:
            raise RuntimeError(f"axon_start_nrt_profile rc={rc}")
        try:
            yield
        finally:
            n = lib.axon_stop_nrt_profile(str(output_dir).encode())
            print(f"ntff profile: {n} file(s) -> {output_dir}", file=sys.stderr)

    mod = types.ModuleType("antenv.axon_hooks")
    mod.get_axon_ntff_profile_hook = lambda: _hook
    mod.set_axon_ntff_profile_hook = lambda h: None
    import antenv
    antenv.axon_hooks = mod
    _sys.modules["antenv.axon_hooks"] = mod


def _is_causal_mask(mask):
    ref = np.where(np.tril(np.ones((S, S), dtype=bool)), 0.0, -1e9)
    return mask.shape == (S, S) and np.array_equal(
        mask.astype(np.float32), ref.astype(np.float32))


def kernel(x, wq, wk, wv, wo, freqs_cos, freqs_sin, mask, _want_trace=False):
    x = np.asarray(x, np.float32)
    mask = np.asarray(mask, np.float32)
    if not _is_causal_mask(mask):
        # general fallback (never hit for the reference's causal mask)
        return _numpy_reference(x, wq, wk, wv, wo, freqs_cos, freqs_sin, mask)

    from concourse.bass_utils import run_bass_kernel_spmd

    if _want_trace:
        _install_ntff_hook()
    if "prog" not in _COMPILED:
        _COMPILED["prog"] = _build_program()
    nc = _COMPILED["prog"]

    in_maps = _host_inputs(np.asarray(x, np.float32), np.asarray(wq, np.float32),
                           np.asarray(wk, np.float32), np.asarray(wv, np.float32),
                           np.asarray(wo, np.float32),
                           np.asarray(freqs_cos, np.float32),
                           np.asarray(freqs_sin, np.float32))
    res = run_bass_kernel_spmd(nc, in_maps, core_ids=list(range(NC)),
                               trace=_want_trace)
    total = np.zeros((D, TOK), np.float32)
    for c in range(NC):
        total += res.results[c]["out"].astype(np.float32)
    out = total.T.reshape(B, S, D).astype(np.float32)
    if _want_trace:
        _COMPILED["last_result"] = res
    return out


def _numpy_reference(x, wq, wk, wv, wo, freqs_cos, freqs_sin, mask):
    import math

    def rope(t):
        t2 = t.reshape(*t.shape[:-1], HD // 2, 2)
        x0, x1 = t2[..., 0], t2[..., 1]
        c = freqs_cos[None, :, None, :]
        s = freqs_sin[None, :, None, :]
        r0 = x0 * c - x1 * s
        r1 = x0 * s + x1 * c
        return np.stack([r0, r1], axis=-1).reshape(t.shape)

    b, s, d = x.shape
    q = (x @ wq.T).reshape(b, s, H, HD)
    k = (x @ wk.T).reshape(b, s, H, HD)
    v = (x @ wv.T).reshape(b, s, H, HD)
    q, k = rope(q), rope(k)
    q = q.transpose(0, 2, 1, 3)
    k = k.transpose(0, 2, 1, 3)
    v = v.transpose(0, 2, 1, 3)
    sc = np.einsum("bhqd,bhkd->bhqk", q, k) / math.sqrt(HD) + mask[None, None]
    sc = sc - sc.max(axis=-1, keepdims=True)
    p = np.exp(sc)
    p /= p.sum(axis=-1, keepdims=True)
    o = np.einsum("bhqk,bhkd->bhqd", p, v).transpose(0, 2, 1, 3).reshape(b, s, d)
    return (o @ wo.T).astype(np.float32)


# revision 36
# speedup vs baseline: 1.1898x; 1.1898x over previous
"""Distributed Trainium2 Bass kernel for causal multi-head attention w/ RoPE.

Problem shapes (hardcoded): B=2, S=2048, D=1024, H=16, HD=64.
Sharding: tensor-parallel over heads — each of 8 cores owns 2 heads
(column slice of wq/wk/wv, row slice of wo). Each core emits its partial
x @ woT contribution; the host sums the 8 partials (the "all-reduce").

Schedule (v3): token-block-outer pipeline. For each 512-token block:
q/k/v projections (bf16 PE, fp32 PSUM), RoPE (PE block-swap matmul + DVE
mul/mul/add), v-transpose into v' = [v | 1] tiles. As soon as block g of
batch b is done, attention group (b, g) runs: per sk-tile, BOTH heads'
scores land in one [128,1024] PSUM tile (h0 cols 0:512, h1 512:1024),
the -1e9 causal mask is accumulated on the PE (tri matmul) for diagonal
tiles, ONE wide exp on ScalarE covers both heads, then two PV matmuls
accumulate [out | denom] per head.

Softmax normalization (hardware-profiled choices): 1/d comes from the
fp32 bit-trick seed (magic - bits(d), as an int32 tensor_scalar) plus a
Newton step shaped as (d*r0 - 2)*r0 = -1/d; woT is host-negated to absorb
the sign. nc.vector.reciprocal costs ~4us per row on HW and
reciprocal_approx_fast returns garbage via this runtime, and Ln/Exp sit
in different activation tables (1.3us reload each way), so DVE int ops
are the only cheap correct path. gpsimd executes ONLY
partition_broadcast: every distinct gpsimd op type swaps in its own Q7
library at ~7.5us per reload, which serialized the whole kernel when
broadcasts, tensor ops and DMA triggers shared the engine. head-1's
normalized tile loopback-DMAs into outT partitions 64:128 (engines
cannot write across partition offsets; DMA can). wo output-chunk matmuls
follow each group; fp32 PSUM is cast to bf16 by DVE/ScalarE copies
(alternating so two casts drain concurrently) and DMA'd out. All bulk
DMA rides sync+scalar queues: every dma_start costs ~0.6-1.0us of issue
time on the issuing engine's sequencer, so gpsimd/vector stay clean for
compute and first-needed tiles alternate between the two queues.
"""

import sys

sys.path.insert(0, "/opt/trn_rl_repo")

import numpy as np
import ml_dtypes

B, S, D, H = 2, 2048, 1024, 16
HD = D // H  # 64
NC = 8
HPC = H // NC  # heads per core = 2
HDC = HPC * HD  # head dims per core = 128
TOK = B * S  # 4096
BF16 = ml_dtypes.bfloat16

_COMPILED = {}


def _build_program():
    import concourse.bass as bass
    import concourse.mybir as mybir
    import concourse.bacc as bacc
    from concourse import tile

    f32 = mybir.dt.float32
    bf16 = mybir.dt.bfloat16
    MULT = mybir.AluOpType.mult
    ADD = mybir.AluOpType.add
    DIV = mybir.AluOpType.divide
    EXP = mybir.ActivationFunctionType.Exp
    LN = mybir.ActivationFunctionType.Ln

    nc = bacc.Bacc("TRN2", target_bir_lowering=False, debug=False, num_devices=NC)

    KT = D // 128  # 8 contraction tiles for projections
    NTB = TOK // 512  # 8 tok blocks of 512
    NG = S // 512  # 4 groups per batch

    xT_d = nc.dram_tensor("xT", [NTB * KT * 128, 512], bf16,
                          kind="ExternalInput").ap()
    wqT_d = nc.dram_tensor("wqT", [D, HDC], bf16, kind="ExternalInput").ap()
    wkT_d = nc.dram_tensor("wkT", [D, HDC], bf16, kind="ExternalInput").ap()
    wvT_d = nc.dram_tensor("wvT", [D, HDC], bf16, kind="ExternalInput").ap()
    woT_d = nc.dram_tensor("woT", [HDC, D], bf16, kind="ExternalInput").ap()
    PT_d = nc.dram_tensor("PT", [HDC, HDC], bf16, kind="ExternalInput").ap()
    cos_d = nc.dram_tensor("cosx", [HDC, S], bf16, kind="ExternalInput").ap()
    sin_d = nc.dram_tensor("sinx", [HDC, S], bf16, kind="ExternalInput").ap()
    tri_d = nc.dram_tensor("tri", [128, 128], bf16, kind="ExternalInput").ap()
    id_d = nc.dram_tensor("ident", [128, 128], bf16, kind="ExternalInput").ap()
    out_d = nc.dram_tensor("out", [D, TOK], bf16, kind="ExternalOutput").ap()

    with tile.TileContext(nc) as tc:
        with (
            tc.tile_pool(name="big", bufs=1) as big,
            tc.tile_pool(name="work", bufs=3) as work,
            tc.tile_pool(name="etp", bufs=10) as etp,
            tc.tile_pool(name="nrm", bufs=3) as nrm,
            tc.tile_pool(name="wop", bufs=6) as wop,
            tc.tile_pool(name="mmp", bufs=2, space="PSUM") as mmp,
            tc.tile_pool(name="scp", bufs=2, space="PSUM") as scp,
            tc.tile_pool(name="pop", bufs=2, space="PSUM") as pop,
        ):
            # ---- input DMAs: first-needed first -------------------------
            # DMA issue cost is ~0.7-1us per dma_start on every engine, so
            # first-needed transfers alternate between the sync and scalar
            # queues and small/late tensors ride scalar (idle early).
            wq = big.tile([128, KT * HDC], bf16, tag="wq")
            wk = big.tile([128, KT * HDC], bf16, tag="wk")
            wv = big.tile([128, KT * HDC], bf16, tag="wv")
            xT = big.tile([128, KT * TOK], bf16, tag="xT")

            def xt_dma(tb, k, eng):
                r0 = (tb * KT + k) * 128
                eng.dma_start(
                    xT[:, k * TOK + tb * 512 : k * TOK + (tb + 1) * 512],
                    xT_d[r0 : r0 + 128, :])

            for k in range(KT):  # first projection block's needs
                (nc.sync if k % 2 else nc.scalar).dma_start(
                    wq[:, k * HDC : (k + 1) * HDC],
                    wqT_d[k * 128 : (k + 1) * 128, :])
            for k in range(KT):
                xt_dma(0, k, nc.scalar if k % 2 else nc.sync)
            for w_sb, w_d in ((wk, wkT_d), (wv, wvT_d)):
                for k in range(KT):
                    nc.sync.dma_start(w_sb[:, k * HDC : (k + 1) * HDC],
                                      w_d[k * 128 : (k + 1) * 128, :])
            for tb in range(1, NTB):
                for k in range(KT):
                    xt_dma(tb, k, nc.sync)

            # scalar queue: PT/ident early (rope + transpose of block 0),
            # then cos/sin, tri, wo
            PT = big.tile([128, 128], bf16, tag="PT")
            nc.scalar.dma_start(PT[:], PT_d[:, :])
            ident = big.tile([128, 128], bf16, tag="ident")
            nc.scalar.dma_start(ident[:], id_d[:, :])
            cosx = big.tile([128, S], bf16, tag="cosx")
            nc.scalar.dma_start(cosx[:], cos_d[:, :])
            sinx = big.tile([128, S], bf16, tag="sinx")
            nc.scalar.dma_start(sinx[:], sin_d[:, :])
            tri = big.tile([128, 128], bf16, tag="tri")
            nc.scalar.dma_start(tri[:], tri_d[:, :])
            wo = big.tile([128, D], bf16, tag="wo")
            nc.scalar.dma_start(wo[:], woT_d[:, :])

            # ---- persistent SBUF state ----------------------------------
            rotq = big.tile([128, TOK], bf16, tag="rotq")
            rotk = big.tile([128, TOK], bf16, tag="rotk")
            # v' tiles: [part, kt, head, 65] with ones in col 64 (set once)
            vp = big.tile([128, TOK // 128, HPC, HD + 1], bf16, tag="vp")
            nc.gpsimd.memset(vp[:, :, :, HD : HD + 1], 1.0)
            outT = [big.tile([128, S], bf16, tag=f"outT{b}", name=f"outT{b}")
                    for b in range(B)]

            def proj_block(tb):
                """projections + RoPE + v' for token block tb (512 toks)"""
                blk = slice(tb * 512, (tb + 1) * 512)
                sblk = slice((tb % NG) * 512, (tb % NG + 1) * 512)
                sbs = []
                for w_sb, nm in ((wq, "q"), (wk, "k"), (wv, "v")):
                    ps = mmp.tile([128, 512], f32, tag="mm", name=f"ps{nm}{tb}")
                    for k in range(KT):
                        nc.tensor.matmul(
                            ps[:],
                            w_sb[:, k * HDC : (k + 1) * HDC],
                            xT[:, k * TOK + tb * 512 : k * TOK + (tb + 1) * 512],
                            start=(k == 0), stop=(k == KT - 1),
                        )
                    sb = work.tile([128, 512], bf16, tag=f"{nm}sb")
                    nc.vector.tensor_copy(sb[:], ps[:])
                    sbs.append(sb)
                qsb, ksb, vsb = sbs
                for src, rotdst in ((qsb, rotq), (ksb, rotk)):
                    pss = mmp.tile([128, 512], f32, tag="mm", name=f"pr{tb}")
                    nc.tensor.matmul(pss[:], PT[:], src[:], start=True, stop=True)
                    t1 = work.tile([128, 512], bf16, tag="t1")
                    nc.vector.tensor_tensor(t1[:], src[:], cosx[:, sblk], MULT)
                    t2 = work.tile([128, 512], bf16, tag="t2")
                    nc.vector.tensor_tensor(t2[:], pss[:], sinx[:, sblk], MULT)
                    nc.vector.tensor_tensor(rotdst[:, blk], t1[:], t2[:], ADD)
                for j in range(4):
                    gkt = tb * 4 + j
                    pst = mmp.tile([128, 128], bf16, tag="mm", name=f"pt{gkt}")
                    nc.tensor.transpose(pst[:], vsb[:, j * 128 : (j + 1) * 128],
                                        ident[:])
                    nc.vector.tensor_copy(
                        vp[:, gkt, :, 0:HD],
                        pst[:].rearrange("p (a i) -> p a i", a=HPC))

            def attention_group(b, g):
                """scores+softmax+PV for sq cols [512g, 512g+512) of batch b"""
                g0 = g * 512
                nkt = 4 * g + 4
                po = [pop.tile([HD + 1, 512], f32, tag="po",
                               name=f"po{b}{g}{h}") for h in range(HPC)]
                for kt in range(nkt):
                    w0 = kt * 128
                    lo = max(0, w0 - g0)
                    sc = scp.tile([128, 1024], f32, tag="sc",
                                  name=f"sc{b}{g}{kt}")
                    # h0: only causal cols; h1: full width (junk below diag
                    # is written, exp'd, and never read by PV)
                    diag = w0 >= g0  # diagonal tile: -1e9 tri mask gets added
                    # h1 covers full width: junk below the diagonal is
                    # written (never read by PV) so the wide exp reads no
                    # stale psum
                    nc.tensor.matmul(
                        sc[:, lo:512],
                        rotk[0:HD, b * S + w0 : b * S + w0 + 128],
                        rotq[0:HD, b * S + g0 + lo : b * S + g0 + 512],
                        start=True, stop=not diag)
                    nc.tensor.matmul(
                        sc[:, 512:1024],
                        rotk[HD : 2 * HD, b * S + w0 : b * S + w0 + 128],
                        rotq[HD : 2 * HD, b * S + g0 : b * S + g0 + 512],
                        start=True, stop=not diag)
                    if diag:
                        nc.tensor.matmul(sc[:, lo : lo + 128], ident[:],
                                         tri[:], start=False, stop=True)
                        nc.tensor.matmul(sc[:, 512 + lo : 512 + lo + 128],
                                         ident[:], tri[:],
                                         start=False, stop=True)
                    et = etp.tile([128, 1024], bf16, tag="et")
                    nc.scalar.activation(et[:, lo:1024], sc[:, lo:1024],
                                         EXP, scale=0.125)
                    nc.tensor.matmul(
                        po[0][:, lo:512], vp[:, b * (S // 128) + kt, 0, :],
                        et[:, lo:512],
                        start=(kt == 0), stop=(kt == nkt - 1))
                    nc.tensor.matmul(
                        po[1][:, lo:512], vp[:, b * (S // 128) + kt, 1, :],
                        et[:, 512 + lo : 1024],
                        start=(kt == 0), stop=(kt == nkt - 1))
                # normalization: out = po[0:64] * (1/po[64]) per column.
                # 1/d via the fp32 bit-trick seed (magic - bits(d), done as
                # NOT(bits(d)) + magic+1 in one int tensor_scalar) plus one
                # Newton step emitted as (d*r0 - 2)*r0 = -r1, so the chain
                # yields -1/d; woT is negated on the host to compensate.
                # gpsimd runs ONLY partition_broadcast (one Q7 library).
                i32 = mybir.dt.int32
                rl = [nrm.tile([1, 512], f32, tag=f"rl{h}", name=f"rl{b}{g}{h}")
                      for h in range(HPC)]
                rt = [nrm.tile([1, 512], f32, tag=f"rt{h}", name=f"rt{b}{g}{h}")
                      for h in range(HPC)]
                r = [nrm.tile([1, 512], f32, tag=f"r{h}", name=f"r{b}{g}{h}")
                     for h in range(HPC)]
                SUB = mybir.AluOpType.subtract
                for h in range(HPC):
                    nc.vector.tensor_scalar(
                        rl[h][0:1, :].bitcast(i32),
                        po[h][HD : HD + 1, :].bitcast(i32),
                        0x7EF311C3, -1, SUB, MULT)
                    nc.vector.tensor_tensor(rt[h][0:1, :],
                                            po[h][HD : HD + 1, :],
                                            rl[h][0:1, :], MULT)
                    nc.vector.scalar_tensor_tensor(
                        r[h][0:1, :], rt[h][0:1, :], 2.0, rl[h][0:1, :],
                        SUB, MULT)
                rb = [nrm.tile([HD, 512], f32, tag="rb", name=f"rb{b}{g}{h}")
                      for h in range(HPC)]
                for h in range(HPC):
                    nc.gpsimd.partition_broadcast(rb[h][:, :], r[h][0:1, :])
                nc.vector.tensor_tensor(outT[b][0:HD, g0 : g0 + 512],
                                        po[0][0:HD, :], rb[0][:, :], MULT)
                oh = nrm.tile([HD, 512], bf16, tag="oh")
                nc.vector.tensor_tensor(oh[:], po[1][0:HD, :], rb[1][:, :],
                                        MULT)
                nc.sync.dma_start(outT[b][HD : 2 * HD, g0 : g0 + 512], oh[:])

            def wo_group(b, g):
                """wo partial for out cols [512g, 512g+512) of batch b"""
                g0 = g * 512
                for o in range(D // 128):
                    psw = mmp.tile([128, 512], f32, tag="mm",
                                   name=f"pw{b}{g}{o}")
                    nc.tensor.matmul(
                        psw[:], wo[:, o * 128 : (o + 1) * 128],
                        outT[b][:, g0 : g0 + 512], start=True, stop=True)
                    wout = wop.tile([128, 512], bf16, tag="wout")
                    if o % 2 == 0:
                        nc.vector.tensor_copy(wout[:], psw[:])
                    else:
                        nc.scalar.copy(wout[:], psw[:])
                    nc.sync.dma_start(
                        out_d[o * 128 : (o + 1) * 128,
                              b * S + g0 : b * S + g0 + 512],
                        wout[:])

            # ---- emission order: software-pipelined schedule ------------
            proj_block(0)
            proj_block(1)
            attention_group(0, 0)
            proj_block(2)
            attention_group(0, 1)
            proj_block(3)
            attention_group(0, 2)
            wo_group(0, 0)
            proj_block(4)
            attention_group(0, 3)
            wo_group(0, 1)
            proj_block(5)
            attention_group(1, 0)
            wo_group(0, 2)
            proj_block(6)
            attention_group(1, 1)
            wo_group(0, 3)
            proj_block(7)
            attention_group(1, 2)
            wo_group(1, 0)
            attention_group(1, 3)
            wo_group(1, 1)
            wo_group(1, 2)
            wo_group(1, 3)

    nc.compile()
    return nc


def _host_inputs(x, wq, wk, wv, wo, freqs_cos, freqs_sin):
    """Build the per-core input maps (all host-side transforms are free)."""
    perm = np.concatenate([np.arange(0, HD, 2), np.arange(1, HD, 2)])  # rot-half
    xTf = x.reshape(TOK, D).T.astype(BF16)  # [D, TOK]
    # chunk-contiguous tiling: row block (tb*KT+k) holds xT[k*128:+128, tb*512:+512]
    xT = np.zeros(((TOK // 512) * (D // 128) * 128, 512), BF16)
    for tb in range(TOK // 512):
        for k in range(D // 128):
            r0 = (tb * (D // 128) + k) * 128
            xT[r0 : r0 + 128, :] = xTf[k * 128 : (k + 1) * 128,
                                       tb * 512 : (tb + 1) * 512]

    # signed block-swap P (per 64-dim head): qs_lo = -q_hi, qs_hi = q_lo
    P = np.zeros((HDC, HDC), np.float32)
    for h in range(HPC):
        base = h * HD
        half = HD // 2
        for i in range(half):
            P[base + i, base + half + i] = -1.0
            P[base + half + i, base + i] = 1.0
    PT = np.ascontiguousarray(P.T).astype(BF16)

    # cos/sin expanded to [HDC, S]; row j within a head uses freq j%32
    half = HD // 2
    idx = np.concatenate([np.arange(half), np.arange(half)])  # [64]
    cos1 = freqs_cos[:, :].T[idx]  # [64, S]
    sin1 = freqs_sin[:, :].T[idx]
    cosx = np.tile(cos1, (HPC, 1)).astype(BF16)  # [128, S]
    sinx = np.tile(sin1, (HPC, 1)).astype(BF16)

    # additive causal mask for the diagonal tile: 0 where sk<=sq, -1e9 else
    tri = np.where(np.triu(np.ones((128, 128), dtype=bool)), 0.0,
                   -1e9).astype(BF16)
    ident = np.eye(128, dtype=np.float32).astype(BF16)

    in_maps = []
    for c in range(NC):
        rows = []
        for h in range(HPC):
            hg = c * HPC + h
            rows.append(hg * HD + perm)
        rows = np.concatenate(rows)
        wq_c = np.ascontiguousarray(wq[rows, :].T).astype(BF16)  # [D, 128]
        wk_c = np.ascontiguousarray(wk[rows, :].T).astype(BF16)
        vrows = np.arange(c * HDC, (c + 1) * HDC)
        wv_c = np.ascontiguousarray(wv[vrows, :].T).astype(BF16)
        # negated: the on-device softmax scale is -1/d (sign from the
        # Newton-step formulation); two sign flips cancel in x @ woT
        wo_c = np.ascontiguousarray(-wo[:, vrows].T).astype(BF16)  # [128, D]
        in_maps.append({
            "xT": xT, "wqT": wq_c, "wkT": wk_c, "wvT": wv_c, "woT": wo_c,
            "PT": PT, "cosx": cosx, "sinx": sinx, "tri": tri,
            "ident": ident,
        })
    return in_maps


def _install_ntff_hook():
    """Provide antenv.axon_hooks (missing in this image) so that
    run_bass_kernel_spmd(trace=True) can capture an NTFF profile via the
    axon PJRT .so — replicates trn_boot._ntff_profile_via_ctypes."""
    import types, ctypes, contextlib, sys as _sys

    if "antenv.axon_hooks" in _sys.modules:
        return
    so_path = "/opt/axon/libaxon_pjrt.so"
    try:
        lib = ctypes.CDLL(so_path)
    except OSError:
        return
    if not hasattr(lib, "axon_start_nrt_profile"):
        return
    lib.axon_start_nrt_profile.argtypes = [ctypes.POINTER(ctypes.c_int64),
                                           ctypes.c_size_t]
    lib.axon_start_nrt_profile.restype = ctypes.c_int64
    lib.axon_stop_nrt_profile.argtypes = [ctypes.c_char_p]
    lib.axon_stop_nrt_profile.restype = ctypes.c_int64

    @contextlib.contextmanager
    def _hook(output_dir, device_ids):
        import jax
        jax.devices()
        if device_ids:
            ids = (ctypes.c_int64 * len(device_ids))(*device_ids)
            rc = lib.axon_start_nrt_profile(ids, len(device_ids))
        else:
            rc = lib.axon_start_nrt_profile(None, 0)
        if rc != 0:
            raise RuntimeError(f"axon_start_nrt_profile rc={rc}")
        try:
            yield
        finally:
            n = lib.axon_stop_nrt_profile(str(output_dir).encode())
            print(f"ntff profile: {n} file(s) -> {output_dir}", file=sys.stderr)

    mod = types.ModuleType("antenv.axon_hooks")
    mod.get_axon_ntff_profile_hook = lambda: _hook
    mod.set_axon_ntff_profile_hook = lambda h: None
    import antenv
    antenv.axon_hooks = mod
    _sys.modules["antenv.axon_hooks"] = mod


def _is_causal_mask(mask):
    ref = np.where(np.tril(np.ones((S, S), dtype=bool)), 0.0, -1e9)
    return mask.shape == (S, S) and np.array_equal(
        mask.astype(np.float32), ref.astype(np.float32))


def kernel(x, wq, wk, wv, wo, freqs_cos, freqs_sin, mask, _want_trace=False):
    x = np.asarray(x, np.float32)
    mask = np.asarray(mask, np.float32)
    if not _is_causal_mask(mask):
        # general fallback (never hit for the reference's causal mask)
        return _numpy_reference(x, wq, wk, wv, wo, freqs_cos, freqs_sin, mask)

    from concourse.bass_utils import run_bass_kernel_spmd

    if _want_trace:
        _install_ntff_hook()
    if "prog" not in _COMPILED:
        _COMPILED["prog"] = _build_program()
    nc = _COMPILED["prog"]

    in_maps = _host_inputs(np.asarray(x, np.float32), np.asarray(wq, np.float32),
                           np.asarray(wk, np.float32), np.asarray(wv, np.float32),
                           np.asarray(wo, np.float32),
                           np.asarray(freqs_cos, np.float32),
                           np.asarray(freqs_sin, np.float32))
    res = run_bass_kernel_spmd(nc, in_maps, core_ids=list(range(NC)),
                               trace=_want_trace)
    total = np.zeros((D, TOK), np.float32)
    for c in range(NC):
        total += res.results[c]["out"].astype(np.float32)
    out = total.T.reshape(B, S, D).astype(np.float32)
    if _want_trace:
        _COMPILED["last_result"] = res
    return out


def _numpy_reference(x, wq, wk, wv, wo, freqs_cos, freqs_sin, mask):
    import math

    def rope(t):
        t2 = t.reshape(*t.shape[:-1], HD // 2, 2)
        x0, x1 = t2[..., 0], t2[..., 1]
        c = freqs_cos[None, :, None, :]
        s = freqs_sin[None, :, None, :]
        r0 = x0 * c - x1 * s
        r1 = x0 * s + x1 * c
        return np.stack([r0, r1], axis=-1).reshape(t.shape)

    b, s, d = x.shape
    q = (x @ wq.T).reshape(b, s, H, HD)
    k = (x @ wk.T).reshape(b, s, H, HD)
    v = (x @ wv.T).reshape(b, s, H, HD)
    q, k = rope(q), rope(k)
    q = q.transpose(0, 2, 1, 3)
    k = k.transpose(0, 2, 1, 3)
    v = v.transpose(0, 2, 1, 3)
    sc = np.einsum("bhqd,bhkd->bhqk", q, k) / math.sqrt(HD) + mask[None, None]
    sc = sc - sc.max(axis=-1, keepdims=True)
    p = np.exp(sc)
    p /= p.sum(axis=-1, keepdims=True)
    o = np.einsum("bhqk,bhkd->bhqd", p, v).transpose(0, 2, 1, 3).reshape(b, s, d)
    return (o @ wo.T).astype(np.float32)


# revision 37
# speedup vs baseline: 1.2313x; 1.0349x over previous
"""Distributed Trainium2 Bass kernel for causal multi-head attention w/ RoPE.

Problem shapes (hardcoded): B=2, S=2048, D=1024, H=16, HD=64.
Sharding: tensor-parallel over heads — each of 8 cores owns 2 heads
(column slice of wq/wk/wv, row slice of wo). Each core emits its partial
x @ woT contribution; the host sums the 8 partials (the "all-reduce").

Schedule (v3): token-block-outer pipeline. For each 512-token block:
q/k/v projections (bf16 PE, fp32 PSUM), RoPE (PE block-swap matmul + DVE
mul/mul/add), v-transpose into v' = [v | 1] tiles. As soon as block g of
batch b is done, attention group (b, g) runs: per sk-tile, BOTH heads'
scores land in one [128,1024] PSUM tile (h0 cols 0:512, h1 512:1024),
the -1e9 causal mask is accumulated on the PE (tri matmul) for diagonal
tiles, ONE wide exp on ScalarE covers both heads, then two PV matmuls
accumulate [out | denom] per head.

Softmax normalization (hardware-profiled choices): 1/d comes from the
fp32 bit-trick seed (magic - bits(d), as an int32 tensor_scalar) plus a
Newton step shaped as (d*r0 - 2)*r0 = -1/d; woT is host-negated to absorb
the sign. nc.vector.reciprocal costs ~4us per row on HW and
reciprocal_approx_fast returns garbage via this runtime, and Ln/Exp sit
in different activation tables (1.3us reload each way), so DVE int ops
are the only cheap correct path. gpsimd executes ONLY
partition_broadcast: every distinct gpsimd op type swaps in its own Q7
library at ~7.5us per reload, which serialized the whole kernel when
broadcasts, tensor ops and DMA triggers shared the engine. head-1's
normalized tile loopback-DMAs into outT partitions 64:128 (engines
cannot write across partition offsets; DMA can). wo output-chunk matmuls
follow each group; fp32 PSUM is cast to bf16 by DVE/ScalarE copies
(alternating so two casts drain concurrently) and DMA'd out. All bulk
DMA rides sync+scalar queues: every dma_start costs ~0.6-1.0us of issue
time on the issuing engine's sequencer, so gpsimd/vector stay clean for
compute and first-needed tiles alternate between the two queues.
"""

import sys

sys.path.insert(0, "/opt/trn_rl_repo")

import numpy as np
import ml_dtypes

B, S, D, H = 2, 2048, 1024, 16
HD = D // H  # 64
NC = 8
HPC = H // NC  # heads per core = 2
HDC = HPC * HD  # head dims per core = 128
TOK = B * S  # 4096
BF16 = ml_dtypes.bfloat16

_COMPILED = {}


def _build_program():
    import concourse.bass as bass
    import concourse.mybir as mybir
    import concourse.bacc as bacc
    from concourse import tile

    f32 = mybir.dt.float32
    bf16 = mybir.dt.bfloat16
    MULT = mybir.AluOpType.mult
    ADD = mybir.AluOpType.add
    DIV = mybir.AluOpType.divide
    EXP = mybir.ActivationFunctionType.Exp
    LN = mybir.ActivationFunctionType.Ln

    nc = bacc.Bacc("TRN2", target_bir_lowering=False, debug=False, num_devices=NC)

    KT = D // 128  # 8 contraction tiles for projections
    NTB = TOK // 512  # 8 tok blocks of 512
    NG = S // 512  # 4 groups per batch

    xT_d = nc.dram_tensor("xT", [NTB * KT * 128, 512], bf16,
                          kind="ExternalInput").ap()
    wqT_d = nc.dram_tensor("wqT", [D, HDC], bf16, kind="ExternalInput").ap()
    wkT_d = nc.dram_tensor("wkT", [D, HDC], bf16, kind="ExternalInput").ap()
    wvT_d = nc.dram_tensor("wvT", [D, HDC], bf16, kind="ExternalInput").ap()
    woT_d = nc.dram_tensor("woT", [HDC, D], bf16, kind="ExternalInput").ap()
    PT_d = nc.dram_tensor("PT", [HDC, HDC], bf16, kind="ExternalInput").ap()
    cos_d = nc.dram_tensor("cosx", [HDC, S], bf16, kind="ExternalInput").ap()
    sin_d = nc.dram_tensor("sinx", [HDC, S], bf16, kind="ExternalInput").ap()
    tri_d = nc.dram_tensor("tri", [128, 128], bf16, kind="ExternalInput").ap()
    id_d = nc.dram_tensor("ident", [128, 128], bf16, kind="ExternalInput").ap()
    out_d = nc.dram_tensor("out", [D, TOK], bf16, kind="ExternalOutput").ap()

    with tile.TileContext(nc) as tc:
        with (
            tc.tile_pool(name="big", bufs=1) as big,
            tc.tile_pool(name="work", bufs=4) as work,
            tc.tile_pool(name="etp", bufs=10) as etp,
            tc.tile_pool(name="nrm", bufs=3) as nrm,
            tc.tile_pool(name="wop", bufs=8) as wop,
            tc.tile_pool(name="mmp", bufs=2, space="PSUM") as mmp,
            tc.tile_pool(name="scp", bufs=2, space="PSUM") as scp,
            tc.tile_pool(name="pop", bufs=2, space="PSUM") as pop,
        ):
            # ---- input DMAs: first-needed first -------------------------
            # DMA issue cost is ~0.7-1us per dma_start on every engine, so
            # first-needed transfers alternate between the sync and scalar
            # queues and small/late tensors ride scalar (idle early).
            wq = big.tile([128, KT * HDC], bf16, tag="wq")
            wk = big.tile([128, KT * HDC], bf16, tag="wk")
            wv = big.tile([128, KT * HDC], bf16, tag="wv")
            xT = big.tile([128, KT * TOK], bf16, tag="xT")

            def xt_dma(tb, k, eng):
                r0 = (tb * KT + k) * 128
                eng.dma_start(
                    xT[:, k * TOK + tb * 512 : k * TOK + (tb + 1) * 512],
                    xT_d[r0 : r0 + 128, :])

            for k in range(KT):  # first projection block's needs
                (nc.sync if k % 2 else nc.scalar).dma_start(
                    wq[:, k * HDC : (k + 1) * HDC],
                    wqT_d[k * 128 : (k + 1) * 128, :])
            for k in range(KT):
                xt_dma(0, k, nc.scalar if k % 2 else nc.sync)
            for w_sb, w_d in ((wk, wkT_d), (wv, wvT_d)):
                for k in range(KT):
                    nc.sync.dma_start(w_sb[:, k * HDC : (k + 1) * HDC],
                                      w_d[k * 128 : (k + 1) * 128, :])
            for tb in range(1, NTB):
                for k in range(KT):
                    xt_dma(tb, k, nc.sync)

            # scalar queue: PT/ident early (rope + transpose of block 0),
            # then cos/sin, tri, wo
            PT = big.tile([128, 128], bf16, tag="PT")
            nc.scalar.dma_start(PT[:], PT_d[:, :])
            ident = big.tile([128, 128], bf16, tag="ident")
            nc.scalar.dma_start(ident[:], id_d[:, :])
            cosx = big.tile([128, S], bf16, tag="cosx")
            nc.scalar.dma_start(cosx[:], cos_d[:, :])
            sinx = big.tile([128, S], bf16, tag="sinx")
            nc.scalar.dma_start(sinx[:], sin_d[:, :])
            tri = big.tile([128, 128], bf16, tag="tri")
            nc.scalar.dma_start(tri[:], tri_d[:, :])
            wo = big.tile([128, D], bf16, tag="wo")
            nc.scalar.dma_start(wo[:], woT_d[:, :])

            # ---- persistent SBUF state ----------------------------------
            rotq = big.tile([128, TOK], bf16, tag="rotq")
            rotk = big.tile([128, TOK], bf16, tag="rotk")
            # v' tiles: [part, kt, head, 65] with ones in col 64 (set once)
            vp = big.tile([128, TOK // 128, HPC, HD + 1], bf16, tag="vp")
            nc.gpsimd.memset(vp[:, :, :, HD : HD + 1], 1.0)
            outT = [big.tile([128, S], bf16, tag=f"outT{b}", name=f"outT{b}")
                    for b in range(B)]

            def proj_block(tb):
                """projections + RoPE + v' for token block tb (512 toks)"""
                blk = slice(tb * 512, (tb + 1) * 512)
                sblk = slice((tb % NG) * 512, (tb % NG + 1) * 512)
                sbs = []
                for w_sb, nm in ((wq, "q"), (wk, "k"), (wv, "v")):
                    ps = mmp.tile([128, 512], f32, tag="mm", name=f"ps{nm}{tb}")
                    for k in range(KT):
                        nc.tensor.matmul(
                            ps[:],
                            w_sb[:, k * HDC : (k + 1) * HDC],
                            xT[:, k * TOK + tb * 512 : k * TOK + (tb + 1) * 512],
                            start=(k == 0), stop=(k == KT - 1),
                        )
                    sb = work.tile([128, 512], bf16, tag=f"{nm}sb")
                    nc.vector.tensor_copy(sb[:], ps[:])
                    sbs.append(sb)
                qsb, ksb, vsb = sbs
                for src, rotdst in ((qsb, rotq), (ksb, rotk)):
                    pss = mmp.tile([128, 512], f32, tag="mm", name=f"pr{tb}")
                    nc.tensor.matmul(pss[:], PT[:], src[:], start=True, stop=True)
                    t1 = work.tile([128, 512], bf16, tag="t1")
                    nc.vector.tensor_tensor(t1[:], src[:], cosx[:, sblk], MULT)
                    t2 = work.tile([128, 512], bf16, tag="t2")
                    nc.vector.tensor_tensor(t2[:], pss[:], sinx[:, sblk], MULT)
                    nc.vector.tensor_tensor(rotdst[:, blk], t1[:], t2[:], ADD)
                for j in range(4):
                    gkt = tb * 4 + j
                    pst = mmp.tile([128, 128], bf16, tag="mm", name=f"pt{gkt}")
                    nc.tensor.transpose(pst[:], vsb[:, j * 128 : (j + 1) * 128],
                                        ident[:])
                    nc.vector.tensor_copy(
                        vp[:, gkt, :, 0:HD],
                        pst[:].rearrange("p (a i) -> p a i", a=HPC))

            def attention_group(b, g):
                """scores+softmax+PV for sq cols [512g, 512g+512) of batch b"""
                g0 = g * 512
                nkt = 4 * g + 4
                po = [pop.tile([HD + 1, 512], f32, tag="po",
                               name=f"po{b}{g}{h}") for h in range(HPC)]
                for kt in range(nkt):
                    w0 = kt * 128
                    lo = max(0, w0 - g0)
                    sc = scp.tile([128, 1024], f32, tag="sc",
                                  name=f"sc{b}{g}{kt}")
                    # h0: only causal cols; h1: full width (junk below diag
                    # is written, exp'd, and never read by PV)
                    diag = w0 >= g0  # diagonal tile: -1e9 tri mask gets added
                    # h1 covers full width: junk below the diagonal is
                    # written (never read by PV) so the wide exp reads no
                    # stale psum
                    nc.tensor.matmul(
                        sc[:, lo:512],
                        rotk[0:HD, b * S + w0 : b * S + w0 + 128],
                        rotq[0:HD, b * S + g0 + lo : b * S + g0 + 512],
                        start=True, stop=not diag)
                    nc.tensor.matmul(
                        sc[:, 512:1024],
                        rotk[HD : 2 * HD, b * S + w0 : b * S + w0 + 128],
                        rotq[HD : 2 * HD, b * S + g0 : b * S + g0 + 512],
                        start=True, stop=not diag)
                    if diag:
                        nc.tensor.matmul(sc[:, lo : lo + 128], ident[:],
                                         tri[:], start=False, stop=True)
                        nc.tensor.matmul(sc[:, 512 + lo : 512 + lo + 128],
                                         ident[:], tri[:],
                                         start=False, stop=True)
                    et = etp.tile([128, 1024], bf16, tag="et")
                    nc.scalar.activation(et[:, lo:1024], sc[:, lo:1024],
                                         EXP, scale=0.125)
                    nc.tensor.matmul(
                        po[0][:, lo:512], vp[:, b * (S // 128) + kt, 0, :],
                        et[:, lo:512],
                        start=(kt == 0), stop=(kt == nkt - 1))
                    nc.tensor.matmul(
                        po[1][:, lo:512], vp[:, b * (S // 128) + kt, 1, :],
                        et[:, 512 + lo : 1024],
                        start=(kt == 0), stop=(kt == nkt - 1))
                # normalization: out = po[0:64] * (1/po[64]) per column.
                # 1/d via the fp32 bit-trick seed (magic - bits(d), done as
                # NOT(bits(d)) + magic+1 in one int tensor_scalar) plus one
                # Newton step emitted as (d*r0 - 2)*r0 = -r1, so the chain
                # yields -1/d; woT is negated on the host to compensate.
                # gpsimd runs ONLY partition_broadcast (one Q7 library).
                i32 = mybir.dt.int32
                rl = [nrm.tile([1, 512], f32, tag=f"rl{h}", name=f"rl{b}{g}{h}")
                      for h in range(HPC)]
                rt = [nrm.tile([1, 512], f32, tag=f"rt{h}", name=f"rt{b}{g}{h}")
                      for h in range(HPC)]
                r = [nrm.tile([1, 512], f32, tag=f"r{h}", name=f"r{b}{g}{h}")
                     for h in range(HPC)]
                SUB = mybir.AluOpType.subtract
                for h in range(HPC):
                    nc.vector.tensor_scalar(
                        rl[h][0:1, :].bitcast(i32),
                        po[h][HD : HD + 1, :].bitcast(i32),
                        0x7EF311C3, -1, SUB, MULT)
                    nc.vector.tensor_tensor(rt[h][0:1, :],
                                            po[h][HD : HD + 1, :],
                                            rl[h][0:1, :], MULT)
                    nc.vector.scalar_tensor_tensor(
                        r[h][0:1, :], rt[h][0:1, :], 2.0, rl[h][0:1, :],
                        SUB, MULT)
                rb = [nrm.tile([HD, 512], f32, tag="rb", name=f"rb{b}{g}{h}")
                      for h in range(HPC)]
                for h in range(HPC):
                    nc.gpsimd.partition_broadcast(rb[h][:, :], r[h][0:1, :])
                nc.vector.tensor_tensor(outT[b][0:HD, g0 : g0 + 512],
                                        po[0][0:HD, :], rb[0][:, :], MULT)
                oh = nrm.tile([HD, 512], bf16, tag="oh")
                nc.vector.tensor_tensor(oh[:], po[1][0:HD, :], rb[1][:, :],
                                        MULT)
                nc.sync.dma_start(outT[b][HD : 2 * HD, g0 : g0 + 512], oh[:])

            def wo_group(b, g):
                """wo partial for out cols [512g, 512g+512) of batch b"""
                g0 = g * 512
                for o in range(D // 128):
                    psw = mmp.tile([128, 512], f32, tag="mm",
                                   name=f"pw{b}{g}{o}")
                    nc.tensor.matmul(
                        psw[:], wo[:, o * 128 : (o + 1) * 128],
                        outT[b][:, g0 : g0 + 512], start=True, stop=True)
                    wout = wop.tile([128, 512], bf16, tag="wout")
                    if o % 2 == 0:
                        nc.vector.tensor_copy(wout[:], psw[:])
                    else:
                        nc.scalar.copy(wout[:], psw[:])
                    nc.sync.dma_start(
                        out_d[o * 128 : (o + 1) * 128,
                              b * S + g0 : b * S + g0 + 512],
                        wout[:])

            # ---- emission order: software-pipelined schedule ------------
            proj_block(0)
            proj_block(1)
            attention_group(0, 0)
            proj_block(2)
            attention_group(0, 1)
            proj_block(3)
            attention_group(0, 2)
            wo_group(0, 0)
            proj_block(4)
            attention_group(0, 3)
            wo_group(0, 1)
            proj_block(5)
            attention_group(1, 0)
            wo_group(0, 2)
            proj_block(6)
            attention_group(1, 1)
            wo_group(0, 3)
            proj_block(7)
            attention_group(1, 2)
            wo_group(1, 0)
            attention_group(1, 3)
            wo_group(1, 1)
            wo_group(1, 2)
            wo_group(1, 3)

    nc.compile()
    return nc


def _host_inputs(x, wq, wk, wv, wo, freqs_cos, freqs_sin):
    """Build the per-core input maps (all host-side transforms are free)."""
    perm = np.concatenate([np.arange(0, HD, 2), np.arange(1, HD, 2)])  # rot-half
    xTf = x.reshape(TOK, D).T.astype(BF16)  # [D, TOK]
    # chunk-contiguous tiling: row block (tb*KT+k) holds xT[k*128:+128, tb*512:+512]
    xT = np.zeros(((TOK // 512) * (D // 128) * 128, 512), BF16)
    for tb in range(TOK // 512):
        for k in range(D // 128):
            r0 = (tb * (D // 128) + k) * 128
            xT[r0 : r0 + 128, :] = xTf[k * 128 : (k + 1) * 128,
                                       tb * 512 : (tb + 1) * 512]

    # signed block-swap P (per 64-dim head): qs_lo = -q_hi, qs_hi = q_lo
    P = np.zeros((HDC, HDC), np.float32)
    for h in range(HPC):
        base = h * HD
        half = HD // 2
        for i in range(half):
            P[base + i, base + half + i] = -1.0
            P[base + half + i, base + i] = 1.0
    PT = np.ascontiguousarray(P.T).astype(BF16)

    # cos/sin expanded to [HDC, S]; row j within a head uses freq j%32
    half = HD // 2
    idx = np.concatenate([np.arange(half), np.arange(half)])  # [64]
    cos1 = freqs_cos[:, :].T[idx]  # [64, S]
    sin1 = freqs_sin[:, :].T[idx]
    cosx = np.tile(cos1, (HPC, 1)).astype(BF16)  # [128, S]
    sinx = np.tile(sin1, (HPC, 1)).astype(BF16)

    # additive causal mask for the diagonal tile: 0 where sk<=sq, -1e9 else
    tri = np.where(np.triu(np.ones((128, 128), dtype=bool)), 0.0,
                   -1e9).astype(BF16)
    ident = np.eye(128, dtype=np.float32).astype(BF16)

    in_maps = []
    for c in range(NC):
        rows = []
        for h in range(HPC):
            hg = c * HPC + h
            rows.append(hg * HD + perm)
        rows = np.concatenate(rows)
        wq_c = np.ascontiguousarray(wq[rows, :].T).astype(BF16)  # [D, 128]
        wk_c = np.ascontiguousarray(wk[rows, :].T).astype(BF16)
        vrows = np.arange(c * HDC, (c + 1) * HDC)
        wv_c = np.ascontiguousarray(wv[vrows, :].T).astype(BF16)
        # negated: the on-device softmax scale is -1/d (sign from the
        # Newton-step formulation); two sign flips cancel in x @ woT
        wo_c = np.ascontiguousarray(-wo[:, vrows].T).astype(BF16)  # [128, D]
        in_maps.append({
            "xT": xT, "wqT": wq_c, "wkT": wk_c, "wvT": wv_c, "woT": wo_c,
            "PT": PT, "cosx": cosx, "sinx": sinx, "tri": tri,
            "ident": ident,
        })
    return in_maps


def _install_ntff_hook():
    """Provide antenv.axon_hooks (missing in this image) so that
    run_bass_kernel_spmd(trace=True) can capture an NTFF profile via the
    axon PJRT .so — replicates trn_boot._ntff_profile_via_ctypes."""
    import types, ctypes, contextlib, sys as _sys

    if "antenv.axon_hooks" in _sys.modules:
        return
    so_path = "/opt/axon/libaxon_pjrt.so"
    try:
        lib = ctypes.CDLL(so_path)
    except OSError:
        return
    if not hasattr(lib, "axon_start_nrt_profile"):
        return
    lib.axon_start_nrt_profile.argtypes = [ctypes.POINTER(ctypes.c_int64),
                                           ctypes.c_size_t]
    lib.axon_start_nrt_profile.restype = ctypes.c_int64
    lib.axon_stop_nrt_profile.argtypes = [ctypes.c_char_p]
    lib.axon_stop_nrt_profile.restype = ctypes.c_int64

    @contextlib.contextmanager
    def _hook(output_dir, device_ids):
        import jax
        jax.devices()
        if device_ids:
            ids = (ctypes.c_int64 * len(device_ids))(*device_ids)
            rc = lib.axon_start_nrt_profile(ids, len(device_ids))
        else:
            rc = lib.axon_start_nrt_profile(None, 0)
        if rc != 0:
            raise RuntimeError(f"axon_start_nrt_profile rc={rc}")
        try:
            yield
        finally:
            n = lib.axon_stop_nrt_profile(str(output_dir).encode())
            print(f"ntff profile: {n} file(s) -> {output_dir}", file=sys.stderr)

    mod = types.ModuleType("antenv.axon_hooks")
    mod.get_axon_ntff_profile_hook = lambda: _hook
    mod.set_axon_ntff_profile_hook = lambda h: None
    import antenv
    antenv.axon_hooks = mod
    _sys.modules["antenv.axon_hooks"] = mod


def _is_causal_mask(mask):
    ref = np.where(np.tril(np.ones((S, S), dtype=bool)), 0.0, -1e9)
    return mask.shape == (S, S) and np.array_equal(
        mask.astype(np.float32), ref.astype(np.float32))


def kernel(x, wq, wk, wv, wo, freqs_cos, freqs_sin, mask, _want_trace=False):
    x = np.asarray(x, np.float32)
    mask = np.asarray(mask, np.float32)
    if not _is_causal_mask(mask):
        # general fallback (never hit for the reference's causal mask)
        return _numpy_reference(x, wq, wk, wv, wo, freqs_cos, freqs_sin, mask)

    from concourse.bass_utils import run_bass_kernel_spmd

    if _want_trace:
        _install_ntff_hook()
    if "prog" not in _COMPILED:
        _COMPILED["prog"] = _build_program()
    nc = _COMPILED["prog"]

    in_maps = _host_inputs(np.asarray(x, np.float32), np.asarray(wq, np.float32),
                           np.asarray(wk, np.float32), np.asarray(wv, np.float32),
                           np.asarray(wo, np.float32),
                           np.asarray(freqs_cos, np.float32),
                           np.asarray(freqs_sin, np.float32))
    res = run_bass_kernel_spmd(nc, in_maps, core_ids=list(range(NC)),
                               trace=_want_trace)
    total = np.zeros((D, TOK), np.float32)
    for c in range(NC):
        total += res.results[c]["out"].astype(np.float32)
    out = total.T.reshape(B, S, D).astype(np.float32)
    if _want_trace:
        _COMPILED["last_result"] = res
    return out


def _numpy_reference(x, wq, wk, wv, wo, freqs_cos, freqs_sin, mask):
    import math

    def rope(t):
        t2 = t.reshape(*t.shape[:-1], HD // 2, 2)
        x0, x1 = t2[..., 0], t2[..., 1]
        c = freqs_cos[None, :, None, :]
        s = freqs_sin[None, :, None, :]
        r0 = x0 * c - x1 * s
        r1 = x0 * s + x1 * c
        return np.stack([r0, r1], axis=-1).reshape(t.shape)

    b, s, d = x.shape
    q = (x @ wq.T).reshape(b, s, H, HD)
    k = (x @ wk.T).reshape(b, s, H, HD)
    v = (x @ wv.T).reshape(b, s, H, HD)
    q, k = rope(q), rope(k)
    q = q.transpose(0, 2, 1, 3)
    k = k.transpose(0, 2, 1, 3)
    v = v.transpose(0, 2, 1, 3)
    sc = np.einsum("bhqd,bhkd->bhqk", q, k) / math.sqrt(HD) + mask[None, None]
    sc = sc - sc.max(axis=-1, keepdims=True)
    p = np.exp(sc)
    p /= p.sum(axis=-1, keepdims=True)
    o = np.einsum("bhqk,bhkd->bhqd", p, v).transpose(0, 2, 1, 3).reshape(b, s, d)
    return (o @ wo.T).astype(np.float32)
